# revision 1
# baseline (speedup 1.0000x reference)
"""Trainium2 Bass kernel for nn_CPA_CCA_block (channel attention + spatial attention + fusion).

Batch-sharded: 8 samples over 8 NeuronCores, replicated weights, zero collectives.
Key tricks:
  - out1/out2 never materialized: w_h @ [out1;out2] folded on host into
    (w_h1@w_beta)@E + (w_h2@w_e)@Esp + (w_h1+w_h2)@x
  - channel-attention softmax logits computed in full fp32 (logits ~N(0,464))
    via PE-transposed x chunks; fat matmuls run in float32r (fp32 with 11-bit
    mantissa, full PE rate); tiny spatial branch in bf16.
  - fusion stage streamed in 480-pixel chunks; Esp kept in (w,h) free order, read
    back through a permuted AP so no output-side transpose is needed.
  - lrelu = (x*0.001) max x on DVE (HW Lrelu activation ignores alpha).
"""
import sys
sys.path.insert(0, '/opt/trn_rl_repo')
import numpy as np
from contextlib import ExitStack

import concourse.bacc as bacc
import concourse.tile as tile
from concourse import mybir
from concourse.bass_utils import run_bass_kernel_spmd
from concourse import bass_isa
import ml_dtypes

F32 = mybir.dt.float32
F32R = mybir.dt.float32r
BF16 = mybir.dt.bfloat16
AF = mybir.ActivationFunctionType
ALU = mybir.AluOpType
AX = mybir.AxisListType

B, C, H, W = 8, 256, 96, 96
HW = H * W
K16 = 16
NEG = 0.001
NCHUNK = 72
FCH = [(k * 480, 480) for k in range(19)] + [(9120, 96)]


def _round_f32r(a):
    b = np.ascontiguousarray(a, dtype=np.float32).view(np.uint32)
    lsb = (b >> np.uint32(12)) & np.uint32(1)
    r = (b + np.uint32(0x7FF) + lsb) & np.uint32(0xFFFFF000)
    return r.view(np.float32)


def _build_program():
    nc = bacc.Bacc("TRN2", target_bir_lowering=False, debug=False)

    def din(name, shape, dt):
        return nc.dram_tensor(name, shape, dt, kind="ExternalInput").ap()

    X = din("x", [C, HW], F32R)
    Wst = din("wst", [C, 18], F32)
    WstR = din("wstr", [C, 18], F32R)
    Whb = din("whb", [2, 128, C], F32R)
    Whx = din("whx", [2, 128, C], F32R)
    Wm1 = din("wm1", [2, 128, C], F32R)
    Wm2 = din("wm2", [2, 128, C], F32R)
    Whm1 = din("whm1", [2, 128, C], F32R)
    Whm2 = din("whm2", [2, 128, C], F32R)
    Whe = din("whe", [2, 128, C], BF16)
    Wf2d = din("wf2d", [4, C], BF16)
    Wf2c = din("wf2c", [4, 1], BF16)
    Wcol = din("wcol", [14, 16], BF16)
    WF2CB = din("wf2cb", [1, 4], F32)
    IDN = din("idn", [128, 128], F32R)

    Y = nc.dram_tensor("y", [C, HW], F32, kind="ExternalOutput").ap()

    def lrelu(out, src):
        nc.scalar.activation(out, src, AF.Prelu, alpha=NEG)

    def dve_lrelu(out, ps, tmp):
        # exact lrelu in 2 DVE ops, single PSUM read each:
        # tmp = 0.999*relu(ps); out = 0.001*ps + tmp
        nc.vector.tensor_scalar(tmp, ps, 0.0, 1.0 - NEG, op0=ALU.max, op1=ALU.mult)
        nc.vector.scalar_tensor_tensor(out, ps, NEG, tmp, op0=ALU.mult, op1=ALU.add)

    with tile.TileContext(nc) as tc, ExitStack() as ctx:
        per = ctx.enter_context(tc.tile_pool(name="per", bufs=1))
        x0 = per.tile([128, HW], F32R, tag="x0")
        x1 = per.tile([128, HW], F32R, tag="x1")
        nc.sync.dma_start(x0, X[0:128, :])
        nc.sync.dma_start(x1, X[128:256, :])
        xs = [x0, x1]

        idn_r = per.tile([128, 128], F32R, tag="idn", name="idn_r")
        nc.sync.dma_start(idn_r, IDN)
        idn = idn_r.bitcast(F32)
        wst, wstr = [], []
        for ct in range(2):
            t = per.tile([128, 18], F32, tag=f"wst{ct}", name=f"wst{ct}")
            nc.sync.dma_start(t, Wst[ct * 128:(ct + 1) * 128, :])
            wst.append(t)
            t2 = per.tile([128, 18], F32R, tag=f"wstr{ct}", name=f"wstr{ct}")
            nc.sync.dma_start(t2, WstR[ct * 128:(ct + 1) * 128, :])
            wstr.append(t2)

        def load_w(name, ap, dt):
            ts = []
            for kt in range(2):
                t = per.tile([128, C], dt, tag=f"{name}{kt}", name=f"{name}{kt}")
                nc.sync.dma_start(t, ap[kt])
                ts.append(t)
            return ts

        whb = load_w("whb", Whb, F32R)
        whx = load_w("whx", Whx, F32R)
        wm1 = load_w("wm1", Wm1, F32R)
        wm2 = load_w("wm2", Wm2, F32R)
        whm1 = load_w("whm1", Whm1, F32R)
        whm2 = load_w("whm2", Whm2, F32R)
        whe = load_w("whe", Whe, BF16)
        wf2d = per.tile([4, C], BF16, tag="wf2d")
        wf2c = per.tile([4, 1], BF16, tag="wf2c")
        wcol = per.tile([14, 16], BF16, tag="wcol")
        nc.sync.dma_start(wf2d, Wf2d)
        nc.sync.dma_start(wf2c, Wf2c)
        nc.sync.dma_start(wcol, Wcol)
        wf2cb = per.tile([96, 4], F32, tag="wf2cb")
        nc.gpsimd.dma_start(out=wf2cb, in_=WF2CB.to_broadcast((96, 4)))

        stats = per.tile([18, HW], F32R, tag="stats")
        Wst2T = per.tile([K16, C], F32R, tag="Wst2T")
        S_sb = per.tile([K16, C], F32R, tag="S_sb")
        sspT_bf = per.tile([96, 96], BF16, tag="sspT")

        # ---------------- stage A: stats rows (f32r) ----------------
        sbLate = ctx.enter_context(tc.tile_pool(name="sbLate", bufs=1))
        pl6 = sbLate.tile([96, 5, 96], F32, tag="pl6")   # cm, bm_raw, bm, bmT, l/sex
        sv4 = sbLate.tile([96, 4], F32, tag="sv4")
        ssp = sbLate.tile([96, 96], F32, tag="ssp")
        f4r = sbLate.tile([4, HW], BF16, tag="f4r")

        with tc.tile_pool(name="psA", bufs=2, space="PSUM") as psA:
            for t in range(18):
                n0 = t * 512
                ps = psA.tile([18, 512], F32, tag="ps")
                nc.tensor.matmul(ps, wstr[0], x0[:, n0:n0 + 512], start=True, stop=False)
                nc.tensor.matmul(ps, wstr[1], x1[:, n0:n0 + 512], start=False, stop=True)
                nc.vector.tensor_copy(stats[:, n0:n0 + 512], ps)

        # ---------------- spatial: max, Col, conv, planes, F4 ----------------
        with tc.tile_pool(name="sbS1", bufs=1) as sbS1:
            col = sbS1.tile([14, HW], BF16, tag="col")
            nc.gpsimd.memset(col, 0.0)
            with tc.tile_pool(name="sbMax", bufs=1) as sbMax:
                max_tmp = sbMax.tile([128, HW], BF16, tag="max_tmp")
                nc.vector.tensor_tensor(max_tmp, x0.bitcast(F32), x1.bitcast(F32), op=ALU.max)
                # channel max: all-reduce across partitions (in place), row 0 -> Col row 10
                nc.gpsimd.partition_all_reduce(max_tmp, max_tmp, channels=128,
                                               reduce_op=bass_isa.ReduceOp.max)
                nc.sync.dma_start(col[10:11, :], max_tmp[0:1, :])

            avg_ap = stats[16:17, :].bitcast(F32)
            for dy in range(-3, 4):
                r = dy + 3
                s0, s1 = max(0, -dy * 96), HW - max(0, dy * 96)
                nc.gpsimd.dma_start(out=col[r:r + 1, s0:s1], in_=avg_ap[:, s0 + dy * 96:s1 + dy * 96])
                if dy != 0:
                    nc.sync.dma_start(out=col[r + 7:r + 8, s0:s1], in_=col[10:11, s0 + dy * 96:s1 + dy * 96])

            c16 = sbS1.tile([16, HW], BF16, tag="c16")
            with tc.tile_pool(name="psCv", bufs=2, space="PSUM") as psCv:
                for t in range(18):
                    n0 = t * 512
                    ps = psCv.tile([16, 512], F32, tag="cps")
                    nc.tensor.matmul(ps, wcol, col[:, n0:n0 + 512], start=True, stop=True)
                    nc.vector.tensor_copy(c16[:, n0:n0 + 512], ps)

            c16_pl = sbS1.tile([96, 16, 96], BF16, tag="c16_pl")
            for r in range(16):
                nc.sync.dma_start(out=c16_pl[:, r, :],
                                  in_=c16[r:r + 1, :].rearrange("q (h w) -> q h w", w=96))
            accs = sbS1.tile([96, 4, 96], F32, tag="accs")
            nc.vector.memset(accs, 0.0)
            f4_pl = sbS1.tile([96, 4, 96], BF16, tag="f4_pl")
            colmap = [[0], [-1, 0, 1], [-2, -1, 0, 1, 2], [-3, -2, -1, 0, 1, 2, 3]]
            rr = 0
            for k, dxs in enumerate(colmap):
                acc = accs[:, k, :]
                first = True
                for dx in dxs:
                    a0, a1 = max(0, -dx), 96 - max(0, dx)
                    src = c16_pl[:, rr, a0 + dx:a1 + dx]
                    if first:
                        nc.vector.tensor_copy(acc[:, a0:a1], src)
                        first = False
                    else:
                        nc.vector.tensor_tensor(acc[:, a0:a1], acc[:, a0:a1], src, op=ALU.add)
                    rr += 1
                lrelu(f4_pl[:, k, :], acc)
            for k in range(4):
                nc.sync.dma_start(out=f4r[k:k + 1, :].rearrange("q (h w) -> q h w", w=96),
                                  in_=f4_pl[:, k, :])

            # Cm directly on planes: j-contraction via per-partition scalars
            cmtmp = sbS1.tile([96, 96], F32, tag="cmtmp")
            nc.vector.tensor_scalar_mul(cmtmp, f4_pl[:, 0, :], wf2cb[:, 0:1])
            for j in range(1, 4):
                nc.vector.scalar_tensor_tensor(cmtmp, f4_pl[:, j, :], wf2cb[:, j:j + 1],
                                               cmtmp, op0=ALU.mult, op1=ALU.add)
            lrelu(pl6[:, 0, :], cmtmp)

            # Bm plane, spatial logits, softmax, SspT (small, before stage C)
            with tc.tile_pool(name="psL2", bufs=2, space="PSUM") as psL2:
                nc.sync.dma_start(out=pl6[:, 1, :],
                                  in_=stats[17:18, :].bitcast(F32).rearrange("q (h w) -> q h w", w=96))
                lrelu(pl6[:, 2, :], pl6[:, 1, :])
                bmT_ps = psL2.tile([96, 96], F32, tag="lps")
                nc.tensor.transpose(bmT_ps, pl6[:, 2, :], idn[0:96, 0:96])
                nc.vector.tensor_copy(pl6[:, 3, :], bmT_ps)
                l_ps = psL2.tile([96, 96], F32, tag="lps")
                nc.tensor.matmul(l_ps, pl6[:, 3, :], pl6[:, 0, :], start=True, stop=True)
                nc.vector.tensor_copy(pl6[:, 4, :], l_ps)
                nc.vector.reduce_max(sv4[:, 0:1], pl6[:, 4, :], axis=AX.X)
                nc.vector.tensor_scalar_mul(sv4[:, 1:2], sv4[:, 0:1], -1.0)
                nc.scalar.activation(ssp, pl6[:, 4, :], AF.Exp, bias=sv4[:, 1:2], scale=1.0)
                nc.vector.reduce_sum(sv4[:, 2:3], ssp, axis=AX.X)
                nc.vector.reciprocal(sv4[:, 3:4], sv4[:, 2:3])
                nc.vector.tensor_scalar_mul(ssp, ssp, sv4[:, 3:4])
                sspT_ps = psL2.tile([96, 96], F32, tag="lps")
                nc.tensor.transpose(sspT_ps, ssp, idn[0:96, 0:96])
                nc.scalar.copy(sspT_bf, sspT_ps)

            # ---------------- stage C: x transposes + fp32 S logits ----------------
            with tc.tile_pool(name="psC", bufs=2, space="PSUM") as psC, \
                 tc.tile_pool(name="psS", bufs=2, space="PSUM") as psS, \
                 tc.tile_pool(name="sbC", bufs=2) as sbC:
                sacc = [psS.tile([128, K16], F32, tag="sacc", name=f"sacc{j}", bufs=2)
                        for j in range(2)]
                for i in range(NCHUNK):
                    n0 = i * 128
                    xt = sbC.tile([128, C], F32, tag="xt", bufs=4)
                    for ct in range(2):
                        pt = psC.tile([128, 128], F32, tag="pt", bufs=3)
                        nc.tensor.transpose(pt, xs[ct][:, n0:n0 + 128].bitcast(F32), idn)
                        (nc.vector.tensor_copy if ct == 0 else nc.scalar.copy)(
                            xt[:, ct * 128:(ct + 1) * 128], pt)
                    pst = psC.tile([128, K16], F32, tag="pst")
                    nc.tensor.matmul(pst, xs[0][:, n0:n0 + 128].bitcast(F32), wst[0][:, 0:K16],
                                     start=True, stop=False)
                    nc.tensor.matmul(pst, xs[1][:, n0:n0 + 128].bitcast(F32), wst[1][:, 0:K16],
                                     start=False, stop=True)
                    st = sbC.tile([128, K16], F32, tag="st", bufs=3)
                    nc.scalar.copy(st, pst)
                    for mt in range(2):
                        nc.tensor.matmul(sacc[mt], xt[:, mt * 128:(mt + 1) * 128], st,
                                         start=(i == 0), stop=(i == NCHUNK - 1))

                s_pre = sbC.tile([K16, C], F32, tag="s_pre", bufs=1)
                for mt in range(2):
                    sl = sbC.tile([128, K16], F32, tag="sl")
                    nc.vector.tensor_copy(sl, sacc[mt])
                    pt2 = psC.tile([K16, 128], F32, tag="pst")
                    nc.tensor.transpose(pt2, sl, idn)
                    nc.scalar.copy(s_pre[:, mt * 128:(mt + 1) * 128], pt2)

                sm4 = sbC.tile([K16, 4], F32, tag="sm4", bufs=1)
                nc.vector.reduce_max(sm4[:, 0:1], s_pre, axis=AX.X)
                nc.vector.tensor_scalar_mul(sm4[:, 1:2], sm4[:, 0:1], -1.0)
                ex = sbC.tile([K16, C], F32, tag="ex", bufs=1)
                nc.scalar.activation(ex, s_pre, AF.Exp, bias=sm4[:, 1:2], scale=1.0)
                nc.vector.reduce_sum(sm4[:, 2:3], ex, axis=AX.X)
                nc.vector.reciprocal(sm4[:, 3:4], sm4[:, 2:3])
                nc.vector.tensor_scalar_mul(S_sb, ex, sm4[:, 3:4])

                # fold: Wst2T = S @ WhbT  (Whb@(St@F) == (Whb@St)@F, K=16 in fusion)
                sT = sbC.tile([128, 2, K16], F32R, tag="sT", bufs=1)
                for mt in range(2):
                    psT2 = psC.tile([128, K16], F32, tag="pst")
                    nc.tensor.transpose(psT2.bitcast(F32R),
                                        S_sb[:, mt * 128:(mt + 1) * 128],
                                        idn_r[0:K16, 0:K16])
                    nc.vector.tensor_copy(sT[:, mt, :], psT2)
                pw = psC.tile([K16, C], F32, tag="pw", bufs=1)
                nc.tensor.matmul(pw, sT[:, 0, :].bitcast(F32R), whb[0], start=True, stop=False)
                nc.tensor.matmul(pw, sT[:, 1, :].bitcast(F32R), whb[1], start=False, stop=True)
                nc.vector.tensor_copy(Wst2T, pw)


        # ---------------- Esp production + fusion ----------------
        with tc.tile_pool(name="sbEsp", bufs=1) as sbEsp:
            espT = [sbEsp.tile([128, HW], BF16, tag=f"espT{ch}", name=f"espT{ch}")
                    for ch in range(2)]

            with tc.tile_pool(name="sbDT", bufs=2) as sbDT, \
                 tc.tile_pool(name="psDT", bufs=2, space="PSUM") as psDT, \
                 tc.tile_pool(name="psE2", bufs=2, space="PSUM") as psE2:
                pe4 = [None, None]
                for wp in range(48):
                    pd2 = psDT.tile([96, 2, C], F32, tag="pd", bufs=2)
                    for wi in range(2):
                        nc.tensor.matmul(pd2[:, wi, :], f4r[:, 2 * wp + wi::96], wf2d,
                                         start=True, stop=True, skip_group_check=True)
                    dt_w = sbDT.tile([96, 2, C], BF16, tag="dt_w", bufs=3)
                    lrelu(dt_w, pd2)
                    for wi in range(2):
                        w = 2 * wp + wi
                        g = w % 4
                        for ch in range(2):
                            if g == 0:
                                pe4[ch] = psE2.tile([128, 4, 96], F32, tag=f"pe{ch}",
                                                    name=f"pe{ch}", bufs=2)
                            nc.tensor.matmul(pe4[ch][:, g, :],
                                             dt_w[:, wi, ch * 128:(ch + 1) * 128],
                                             sspT_bf, start=True, stop=True,
                                             skip_group_check=True)
                            if g == 3:
                                nc.vector.tensor_copy(espT[ch][:, (w - 3) * 96:(w + 1) * 96],
                                                      pe4[ch])

            # ------------- fusion (streamed 480-pixel chunks) -------------
            with tc.tile_pool(name="psF", bufs=8, space="PSUM") as psF, \
                 tc.tile_pool(name="sbFu", bufs=2) as sbFu:
                esp_v = [espT[ch].rearrange("p (w h) -> p h w", h=96) for ch in range(2)]
                for (n0, n) in FCH:
                    h0, hn = n0 // 96, n // 96
                    hh_c = [sbFu.tile([128, 480], F32R, tag=f"h{j}", name=f"h{j}", bufs=2)
                            for j in range(2)]
                    m_c = [sbFu.tile([128, 480], F32R, tag=f"m{j}", name=f"m{j}", bufs=2)
                           for j in range(2)]
                    for mt in range(2):
                        ms = slice(mt * 128, (mt + 1) * 128)
                        ps = psF.tile([128, 480], F32, tag="ps", name=f"psH{mt}")
                        nc.tensor.matmul(ps[:, 0:n], Wst2T[:, ms], stats[0:K16, n0:n0 + n],
                                         start=True, stop=False)
                        for kt2 in range(2):
                            nc.tensor.matmul(ps[:, 0:n], whe[kt2][:, ms],
                                             esp_v[kt2][:, h0:h0 + hn, :], start=False, stop=False)
                        nc.tensor.matmul(ps[:, 0:n], whx[0][:, ms], x0[:, n0:n0 + n], start=False, stop=False)
                        nc.tensor.matmul(ps[:, 0:n], whx[1][:, ms], x1[:, n0:n0 + n], start=False, stop=True)
                        lrelu(hh_c[mt][:, 0:n], ps[:, 0:n])
                    for mt in range(2):
                        ms = slice(mt * 128, (mt + 1) * 128)
                        ps = psF.tile([128, 480], F32, tag="ps", name=f"psM{mt}")
                        nc.tensor.matmul(ps[:, 0:n], wm1[0][:, ms], hh_c[0][:, 0:n], start=True, stop=False)
                        nc.tensor.matmul(ps[:, 0:n], wm1[1][:, ms], hh_c[1][:, 0:n], start=False, stop=False)
                        nc.tensor.matmul(ps[:, 0:n], wm2[0][:, ms], x0[:, n0:n0 + n], start=False, stop=False)
                        nc.tensor.matmul(ps[:, 0:n], wm2[1][:, ms], x1[:, n0:n0 + n], start=False, stop=True)
                        nc.scalar.activation(m_c[mt][:, 0:n], ps[:, 0:n], AF.Sigmoid)
                    for mt in range(2):
                        ms = slice(mt * 128, (mt + 1) * 128)
                        ps = psF.tile([128, 480], F32, tag="ps", name=f"psO{mt}")
                        nc.tensor.matmul(ps[:, 0:n], whm1[0][:, ms], hh_c[0][:, 0:n], start=True, stop=False)
                        nc.tensor.matmul(ps[:, 0:n], whm1[1][:, ms], hh_c[1][:, 0:n], start=False, stop=False)
                        nc.tensor.matmul(ps[:, 0:n], whm2[0][:, ms], m_c[0][:, 0:n], start=False, stop=False)
                        nc.tensor.matmul(ps[:, 0:n], whm2[1][:, ms], m_c[1][:, 0:n], start=False, stop=True)
                        oc = sbFu.tile([128, 480], F32, tag="oc")
                        lrelu(oc[:, 0:n], ps[:, 0:n])
                        nc.sync.dma_start(Y[mt * 128:(mt + 1) * 128, n0:n0 + n], oc[:, 0:n])

    if not nc.is_finalized():
        nc.finalize()
    return nc


def _host_weights(w_f, w_beta, w1, w3, w5, w7, w_a2b, w_f2c, w_f2d, w_e, w_h, w_m, w_hm):
    bf = ml_dtypes.bfloat16
    wst = np.concatenate([w_f.T, w_a2b.T, np.full((C, 1), 1.0 / C, np.float32)], axis=1).astype(np.float32)

    def kt(mat):
        return _round_f32r(np.ascontiguousarray(mat.reshape(2, 128, -1)))

    w_h1, w_h2 = w_h[:, :C], w_h[:, C:]
    wcol = np.zeros((14, 16), np.float32)
    colbase = [0, 1, 4, 9]
    for ki, wk in enumerate([w1, w3, w5, w7]):
        p = (wk.shape[2] - 1) // 2
        for ci in range(2):
            for dy in range(-p, p + 1):
                for dx in range(-p, p + 1):
                    wcol[ci * 7 + dy + 3, colbase[ki] + dx + p] = wk[0, ci, dy + p, dx + p]
    return dict(
        wst=wst, wstr=_round_f32r(wst),
        whb=kt((w_h1 @ w_beta).T), whx=kt((w_h1 + w_h2).T),
        wm1=kt(w_m[:, :C].T), wm2=kt(w_m[:, C:].T),
        whm1=kt(w_hm[:, :C].T), whm2=kt(w_hm[:, C:].T),
        whe=np.ascontiguousarray((w_h2 @ w_e).T.reshape(2, 128, C)).astype(bf),
        wf2d=w_f2d.T.astype(bf), wf2c=w_f2c.T.astype(bf), wcol=wcol.astype(bf),
        wf2cb=np.ascontiguousarray(w_f2c.astype(np.float32)),
        idn=np.eye(128, dtype=np.float32),
    )


_NC_CACHE = {}


def kernel(x, w_f, w_beta, w1, w3, w5, w7, w_a2b, w_f2c, w_f2d, w_e, w_h, w_m, w_hm,
           _trace=False):
    if "nc" not in _NC_CACHE:
        _NC_CACHE["nc"] = _build_program()
    nc = _NC_CACHE["nc"]

    args = [np.asarray(a, np.float32) for a in
            (w_f, w_beta, w1, w3, w5, w7, w_a2b, w_f2c, w_f2d, w_e, w_h, w_m, w_hm)]
    wts = _host_weights(*args)
    xr = _round_f32r(np.asarray(x, np.float32).reshape(B, C, HW))
    in_maps = [dict(wts, x=np.ascontiguousarray(xr[i])) for i in range(B)]

    kw = dict(trace=True, trace_cores=[0]) if _trace else {}
    r = run_bass_kernel_spmd(nc, in_maps, list(range(B)), **kw)
    out = np.stack([r.results[i]["y"].reshape(C, H, W) for i in range(B)])
    if _trace:
        kernel._last = r
    return out

